# revision 22
# baseline (speedup 1.0000x reference)
"""GNN message-passing (gather + pos-selected affine + segment mean) on 8 TRN2 cores.

Strategy: shard by destination node (2500 nodes/core; each core handles the edges
targeting its nodes — no cross-core reduction needed). All FP math on device:
  - dma_gather pulls feat[src] rows (1KB each) from the replicated HBM feat table.
  - The per-edge affine folds to per-node form: out[n] = (SL[n]*sig(l) + SR[n]*sig(r)
    + cntL[n]*lb + cntR[n]*rb) / max(cnt,1), where SL/SR are segment sums of the raw
    gathered rows split by pos. Segment sums run as one-hot matmuls (float32r, full
    PE rate) accumulating in PSUM over <=128-node windows.
Host does index-only preprocessing: partitions edges per core, bins nodes into
<=128-node windows with 512-edge-per-stream quotas, builds gather-index / relative
node-id arrays and integer degree counts, and un-permutes output rows at the end.
"""
import sys

_REPO = '/opt/trn_rl_repo'
if _REPO not in sys.path:
    sys.path.insert(0, _REPO)

from contextlib import ExitStack

import numpy as np
import concourse.bacc as bacc
import concourse.mybir as mybir
from concourse.bass_utils import run_bass_kernel_spmd
from concourse.library_config import mlp

N_NODES = 20000
N_CORES = 8
NPC = N_NODES // N_CORES      # nodes per core
EV = 256                      # flattened feature (2 * 128)
Q = 512                       # slots per window per stream (4 tiles of 128)
RD = 16                       # fp32r-converted data tile ring depth
RM = 32                       # one-hot M tile ring depth

TRACE = False
LAST_EXEC_NS = None
LAST_RESULT = None

_cache = {}


# ───────────────────────── device program ─────────────────────────

def _build(nb, with_bias):
    f32 = mybir.dt.float32
    f32r = mybir.dt.float32r
    i16 = mybir.dt.int16
    AF = mybir.ActivationFunctionType
    OP = mybir.AluOpType

    P = (nb + 3) // 4                       # gather chunks per stream
    wlist = [list(range(4 * p, min(4 * p + 4, nb))) for p in range(P)]
    S = nb * Q                              # slots per stream

    nc = bacc.Bacc("TRN2", target_bir_lowering=False, num_swdge_queues=4)

    feat = nc.dram_tensor("feat", [N_NODES, EV], f32, kind="ExternalInput")
    gidxl = nc.dram_tensor("gidxl", [128, S // 16], i16, kind="ExternalInput")
    gidxr = nc.dram_tensor("gidxr", [128, S // 16], i16, kind="ExternalInput")
    rell = nc.dram_tensor("rell", [128, S // 128], f32, kind="ExternalInput")
    relr = nc.dram_tensor("relr", [128, S // 128], f32, kind="ExternalInput")
    iota = nc.dram_tensor("iota", [128, 128], f32, kind="ExternalInput")
    rawl = nc.dram_tensor("rawl", [128, EV], f32, kind="ExternalInput")
    rawr = nc.dram_tensor("rawr", [128, EV], f32, kind="ExternalInput")
    cnt = nc.dram_tensor("cnt", [128, nb], f32, kind="ExternalInput")
    if with_bias:
        lbt = nc.dram_tensor("lbt", [128, EV], f32, kind="ExternalInput")
        rbt = nc.dram_tensor("rbt", [128, EV], f32, kind="ExternalInput")
        cntl = nc.dram_tensor("cntl", [128, nb], f32, kind="ExternalInput")
        cntr = nc.dram_tensor("cntr", [128, nb], f32, kind="ExternalInput")
    out = nc.dram_tensor("out", [nb * 128, EV], f32, kind="ExternalOutput")

    # dry pass: pt (PE tiles done) thresholds after each window's matmuls
    pt_L = {}
    pt_R = {}
    g = 0
    for p in range(P):
        for w in wlist[p]:
            g += 4
            pt_L[w] = g
        for w in wlist[p]:
            g += 4
            pt_R[w] = g
    # cv (fp32r converts done) thresholds after each chunk block, in Act order
    cv_L = {}
    cv_R = {}
    c = 0
    for p in range(P):
        c += 4 * len(wlist[p])
        cv_L[p] = c
        c += 4 * len(wlist[p])
        cv_R[p] = c

    n_in_dma = 6 + (4 if with_bias else 0)

    # dma_gather is limited to 1024 idxs (64 data descriptors per SDMA engine).
    # Sub-gathers round-robin across all 4 SWDGE queues (per-queue desc rings
    # serialize; spreading keeps 4 in flight). One sem per (stream, queue).
    GS = 8  # tiles per sub-gather
    gq_of = {}      # (stream, p, s0//GS) -> queue
    gcum_of = {}    # (stream, p) -> list of (queue, cum_count_after_chunk)
    qcnt = {('l', j): 0 for j in range(4)} | {('r', j): 0 for j in range(4)}
    sctr = {'l': 0, 'r': 0}
    for p in range(P):
        nt = 4 * len(wlist[p])
        for st in ('l', 'r'):
            for si, s0 in enumerate(range(0, nt, GS)):
                # reuse distance of 4 sub-gathers (2 chunks) per sem; chunk
                # p's gathers are gated on Act consuming chunk p-2, so no two
                # DMAs are ever in flight on one sem. R offset by 2 so both
                # streams cover all 4 queues each chunk pair.
                qq = (sctr[st] + (0 if st == 'l' else 2)) % 4
                sctr[st] += 1
                gq_of[(st, p, si)] = qq
                qcnt[(st, qq)] += 1
                gcum_of[(st, p, si)] = (qq, qcnt[(st, qq)])

    with ExitStack() as ctx:
        block = ctx.enter_context(nc.Block())
        sb = lambda name, shape, dt: ctx.enter_context(
            nc.sbuf_tensor(name, shape, dt))
        sem = lambda name: ctx.enter_context(nc.semaphore(name))

        gidxl_sb = sb("gidxl_sb", [128, S // 16], i16)
        gidxr_sb = sb("gidxr_sb", [128, S // 16], i16)
        rell_sb = sb("rell_sb", [128, S // 128], f32)
        relr_sb = sb("relr_sb", [128, S // 128], f32)
        iota_sb = sb("iota_sb", [128, 128], f32)
        rawl_sb = sb("rawl_sb", [128, EV], f32)
        rawr_sb = sb("rawr_sb", [128, EV], f32)
        sigl_sb = sb("sigl_sb", [128, EV], f32)
        sigr_sb = sb("sigr_sb", [128, EV], f32)
        cnt_sb = sb("cnt_sb", [128, nb], f32)
        cntm_sb = sb("cntm_sb", [128, nb], f32)
        recip_sb = sb("recip_sb", [128, nb], f32)
        bufl = [sb("bufl0", [128, 16, EV], f32), sb("bufl1", [128, 16, EV], f32)]
        bufr = [sb("bufr0", [128, 16, EV], f32), sb("bufr1", [128, 16, EV], f32)]
        data_r = sb("data_r", [128, RD, EV], f32r)
        m_sb = sb("m_sb", [128, RM, 128], f32r)
        slr_ring = sb("slr_ring", [128, 4, EV], f32)
        out_ring = sb("out_ring", [128, 4, EV], f32)
        tmp_sb = sb("tmp_sb", [128, EV], f32)
        if with_bias:
            lbt_sb = sb("lbt_sb", [128, EV], f32)
            rbt_sb = sb("rbt_sb", [128, EV], f32)
            cntl_sb = sb("cntl_sb", [128, nb], f32)
            cntr_sb = sb("cntr_sb", [128, nb], f32)
            cntlr_sb = sb("cntlr_sb", [128, nb], f32)
            cntrr_sb = sb("cntrr_sb", [128, nb], f32)

        # one PSUM bank ([128, 512] f32) per window slot; first 256 cols used
        psuml = [ctx.enter_context(nc.psum_tensor(f"psl{i}", [128, 512], f32))
                 for i in range(4)]
        psumr = [ctx.enter_context(nc.psum_tensor(f"psr{i}", [128, 512], f32))
                 for i in range(4)]

        io = sem("io")
        gsl = [sem(f"gsl{j}") for j in range(4)]
        gsr = [sem(f"gsr{j}") for j in range(4)]
        cv_sem = sem("cv")
        ms = sem("ms")
        pt = sem("pt")
        as_ = sem("as_")
        us = sem("us")
        af = sem("af")
        od = [sem("od0"), sem("od1"), sem("od2"), sem("od3")]
        rp = sem("rp")
        sg = sem("sg")
        aseq = sem("aseq")
        gix = sem("gix")
        wm = sem("wm")
        wmd = sem("wmd")

        @block.sync
        def _(sync):
            sync.dma_start(gidxl_sb[:], gidxl[:]).then_inc(gix, 16)
            sync.dma_start(gidxr_sb[:], gidxr[:]).then_inc(gix, 16)
            sync.dma_start(rell_sb[:], rell[:]).then_inc(io, 16)
            sync.dma_start(relr_sb[:], relr[:]).then_inc(io, 16)
            sync.dma_start(iota_sb[:], iota[:]).then_inc(io, 16)
            sync.dma_start(rawl_sb[:], rawl[:]).then_inc(io, 16)
            sync.dma_start(rawr_sb[:], rawr[:]).then_inc(io, 16)
            sync.dma_start(cnt_sb[:], cnt[:]).then_inc(io, 16)
            if with_bias:
                sync.dma_start(lbt_sb[:], lbt[:]).then_inc(io, 16)
                sync.dma_start(rbt_sb[:], rbt[:]).then_inc(io, 16)
                sync.dma_start(cntl_sb[:], cntl[:]).then_inc(io, 16)
                sync.dma_start(cntr_sb[:], cntr[:]).then_inc(io, 16)
            for w in range(nb):
                sync.wait_ge(af, w + 1)
                sync.dma_start(
                    out[w * 128:(w + 1) * 128, :], out_ring[:, w % 4, :]
                ).then_inc(od[w % 4], 16)
            for j in range(4):
                nd = len([w for w in range(nb) if w % 4 == j])
                sync.wait_ge(od[j], 16 * nd)

        @block.gpsimd
        def _(gpsimd):
            gpsimd.load_library(mlp)
            gpsimd.wait_ge(gix, 32)
            for p in range(P):
                nt = 4 * len(wlist[p])
                c0 = 32 * wlist[p][0]
                ncol = 8 * nt
                if p >= 2:
                    gpsimd.wait_ge(cv_sem, cv_L[p - 2])
                for si, s0 in enumerate(range(0, nt, GS)):
                    st = min(GS, nt - s0)
                    qq = gq_of[('l', p, si)]
                    gpsimd.dma_gather(
                        bufl[p % 2][:, s0:s0 + st, :], feat[:],
                        gidxl_sb[:, c0 + 8 * s0:c0 + 8 * (s0 + st)],
                        128 * st, 128 * st, EV, queue_num=qq,
                    ).then_inc(gsl[qq], 16)
                if p >= 2:
                    gpsimd.wait_ge(cv_sem, cv_R[p - 2])
                for si, s0 in enumerate(range(0, nt, GS)):
                    st = min(GS, nt - s0)
                    qq = gq_of[('r', p, si)]
                    gpsimd.dma_gather(
                        bufr[p % 2][:, s0:s0 + st, :], feat[:],
                        gidxr_sb[:, c0 + 8 * s0:c0 + 8 * (s0 + st)],
                        128 * st, 128 * st, EV, queue_num=qq,
                    ).then_inc(gsr[qq], 16)


        @block.scalar
        def _(scalar):
            scalar.wait_ge(io, 16 * n_in_dma)
            scalar.activation(sigl_sb[:], rawl_sb[:], AF.Sigmoid).then_inc(sg, 1)
            scalar.activation(sigr_sb[:], rawr_sb[:], AF.Sigmoid).then_inc(sg, 1)
            scalar.wait_ge(rp, 2)
            cvc = 0
            for p in range(P):
                nt = 4 * len(wlist[p])
                for t0 in range(0, nt, 4):
                    si = t0 // GS
                    if t0 % GS == 0:
                        jq, cnt_ = gcum_of[('l', p, si)]
                        scalar.wait_ge(gsl[jq], 16 * cnt_)
                    if cvc + 4 > RD:
                        scalar.wait_ge(pt, cvc + 4 - RD)
                    scalar.copy(
                        data_r[:, cvc % RD:cvc % RD + 4, :],
                        bufl[p % 2][:, t0:t0 + 4, :],
                    ).then_inc(cv_sem, 4)
                    cvc += 4
                for t0 in range(0, nt, 4):
                    si = t0 // GS
                    if t0 % GS == 0:
                        jq, cnt_ = gcum_of[('r', p, si)]
                        scalar.wait_ge(gsr[jq], 16 * cnt_)
                    if cvc + 4 > RD:
                        scalar.wait_ge(pt, cvc + 4 - RD)
                    scalar.copy(
                        data_r[:, cvc % RD:cvc % RD + 4, :],
                        bufr[p % 2][:, t0:t0 + 4, :],
                    ).then_inc(cv_sem, 4)
                    cvc += 4
                for w in wlist[p]:
                    if w >= 4:
                        scalar.wait_ge(af, w - 3)
                    scalar.wait_ge(pt, pt_L[w])
                    scalar.activation(
                        slr_ring[:, w % 4, :], psuml[w % 4][:, 0:EV], AF.Copy,
                        bias=0.0, scale=recip_sb[:, w:w + 1],
                    ).then_inc(as_, 1)

        @block.vector
        def _(vector):
            vector.wait_ge(io, 16 * n_in_dma)
            vector.tensor_scalar(cntm_sb[:], cnt_sb[:], 1.0, None, OP.max
                                 ).then_inc(rp, 1)
            vector.wait_ge(rp, 1)
            vector.reciprocal(recip_sb[:], cntm_sb[:]).then_inc(rp, 1)
            if with_bias:
                vector.wait_ge(rp, 2)
                vector.tensor_mul(cntlr_sb[:], cntl_sb[:], recip_sb[:]
                                  ).then_inc(rp, 1)
                vector.tensor_mul(cntrr_sb[:], cntr_sb[:], recip_sb[:]
                                  ).then_inc(rp, 1)

            ai = [0]
            av = [0]

            def affine(w):
                n = ai[0]
                vector.wait_ge(pt, pt_R[w])
                vector.wait_ge(as_, w + 1)
                vector.wait_ge(sg, 2)
                if w >= 4:
                    vector.wait_ge(od[w % 4], 16 * (w // 4))
                # u = (PSUM_R * recip) ⊙ sigR  -> out_ring slot
                vector.scalar_tensor_tensor(
                    out_ring[:, w % 4, :], psumr[w % 4][:, 0:EV],
                    recip_sb[:, w:w + 1], sigr_sb[:],
                    OP.mult, OP.mult,
                ).then_inc(us, 1)
                # v = SLr ⊙ sigL -> tmp (WAR vs previous window's add via af)
                if n >= 1:
                    vector.wait_ge(af, n)
                vector.tensor_mul(tmp_sb[:], slr_ring[:, w % 4, :], sigl_sb[:]
                                  ).then_inc(aseq, 1)
                av[0] += 1
                # out = u + v (same-engine RAW: both writes must retire)
                vector.wait_ge(us, n + 1)
                vector.wait_ge(aseq, av[0])
                last = vector.tensor_add(
                    out_ring[:, w % 4, :], tmp_sb[:], out_ring[:, w % 4, :])
                if with_bias:
                    last.then_inc(aseq, 1)
                    av[0] += 1
                    vector.wait_ge(rp, 4)
                    vector.wait_ge(aseq, av[0])
                    vector.scalar_tensor_tensor(
                        out_ring[:, w % 4, :], lbt_sb[:], cntlr_sb[:, w:w + 1],
                        out_ring[:, w % 4, :], OP.mult, OP.add
                    ).then_inc(aseq, 1)
                    av[0] += 1
                    vector.wait_ge(aseq, av[0])
                    last = vector.scalar_tensor_tensor(
                        out_ring[:, w % 4, :], rbt_sb[:], cntrr_sb[:, w:w + 1],
                        out_ring[:, w % 4, :], OP.mult, OP.add)
                last.then_inc(af, 1)
                ai[0] = n + 1

            mcv = [0]
            iota_b = iota_sb[:, None, :].broadcast_to([128, 4, 128])

            def mbuild(rel_tab, p, t0):
                mc = mcv[0]
                col = 4 * wlist[p][0] + t0
                if mc + 4 > RM:
                    vector.wait_ge(pt, mc + 4 - RM)
                rel_b = rel_tab[:, col:col + 4][:, :, None].broadcast_to(
                    [128, 4, 128])
                vector.tensor_tensor(
                    m_sb[:, mc % RM:mc % RM + 4, :], iota_b, rel_b,
                    OP.is_equal,
                ).then_inc(ms, 4)

            for p in range(P):
                nt = 4 * len(wlist[p])
                for t0 in range(0, nt, 4):
                    mbuild(rell_sb, p, t0)
                    mcv[0] += 4
                if p >= 1:
                    for w in wlist[p - 1]:
                        affine(w)
                for t0 in range(0, nt, 4):
                    mbuild(relr_sb, p, t0)
                    mcv[0] += 4
            for w in wlist[P - 1]:
                affine(w)

        @block.tensor
        def _(tensor):
            gg = 0
            for p in range(P):
                for w in wlist[p]:
                    if w >= 4:
                        tensor.wait_ge(as_, w - 3)
                    for q_ in range(4):
                        tensor.wait_ge(cv_sem, gg - gg % 4 + 4)
                        tensor.wait_ge(ms, gg - gg % 4 + 4)
                        tensor.matmul(
                            psuml[w % 4][:, 0:EV], m_sb[:, gg % RM, :],
                            data_r[:, gg % RD, :],
                            start=(q_ == 0), stop=(q_ == 3),
                        ).then_inc(pt, 1)
                        gg += 1
                for w in wlist[p]:
                    if w >= 4:
                        tensor.wait_ge(us, w - 3)
                    for q_ in range(4):
                        tensor.wait_ge(cv_sem, gg - gg % 4 + 4)
                        tensor.wait_ge(ms, gg - gg % 4 + 4)
                        tensor.matmul(
                            psumr[w % 4][:, 0:EV], m_sb[:, gg % RM, :],
                            data_r[:, gg % RD, :],
                            start=(q_ == 0), stop=(q_ == 3),
                        ).then_inc(pt, 1)
                        gg += 1

    nc.compile()
    return nc


# ───────────────────────── host preprocessing ─────────────────────────

def _wrap16(idx):
    a = idx.reshape(-1, 16).T.astype(np.int16)
    return np.ascontiguousarray(np.tile(a, (8, 1)))


def _bin_nodes(dl_k, pos_k):
    degL = np.bincount(dl_k[pos_k == 0], minlength=NPC)
    degR = np.bincount(dl_k[pos_k == 1], minlength=NPC)
    deg = degL + degR
    active = np.where(deg > 0)[0]
    order = active[np.argsort(-deg[active], kind='stable')]

    node_bin = np.full(NPC, -1, np.int64)
    node_rel = np.full(NPC, -1, np.int64)
    bins = []  # [node_count, l_sum, r_sum]
    for n in order:
        dn_l, dn_r = int(degL[n]), int(degR[n])
        for bi, b in enumerate(bins):
            if b[0] < 128 and b[1] + dn_l <= Q and b[2] + dn_r <= Q:
                node_bin[n] = bi
                node_rel[n] = b[0]
                b[0] += 1
                b[1] += dn_l
                b[2] += dn_r
                break
        else:
            node_bin[n] = len(bins)
            node_rel[n] = 0
            bins.append([1, dn_l, dn_r])
    return len(bins), (degL, degR, deg, active, node_bin, node_rel)


def _core_arrays(src_k, dl_k, pos_k, state, nb):
    degL, degR, deg, active, node_bin, node_rel = state
    S = nb * Q
    res = {}
    for name, sel in (("l", pos_k == 0), ("r", pos_k == 1)):
        es = np.where(sel)[0]
        b = node_bin[dl_k[es]]
        o = np.lexsort((src_k[es], b))
        se = es[o]
        bb = node_bin[dl_k[se]]
        starts = np.searchsorted(bb, np.arange(nb))
        rank = np.arange(len(se)) - starts[bb]
        slot = bb * Q + rank
        gidx = np.zeros(S, np.int64)
        rel = np.full(S, -1.0, np.float32)
        gidx[slot] = src_k[se]
        rel[slot] = node_rel[dl_k[se]]
        res["gidx" + name] = _wrap16(gidx)
        res["rel" + name] = np.ascontiguousarray(rel.reshape(S // 128, 128).T)
    for nm, d in (("cnt", deg), ("cntl", degL), ("cntr", degR)):
        t = np.zeros((128, nb), np.float32)
        t[node_rel[active], node_bin[active]] = d[active]
        res[nm] = t
    rows = node_bin[active] * 128 + node_rel[active]
    return res, active, rows


# ───────────────────────── entry point ─────────────────────────

def kernel(feat, decomp_l, decomp_r, lb, rb, src, dst, pos):
    global LAST_EXEC_NS, LAST_RESULT
    feat = np.asarray(feat, np.float32)
    src = np.asarray(src)
    dst = np.asarray(dst)
    pos = np.asarray(pos)
    feat2 = np.ascontiguousarray(feat.reshape(N_NODES, EV))
    with_bias = bool(np.any(np.asarray(lb)) or np.any(np.asarray(rb)))

    states = []
    nb = 0
    for k in range(N_CORES):
        lo = k * NPC
        m = (dst >= lo) & (dst < lo + NPC)
        src_k = src[m].astype(np.int64)
        dl_k = (dst[m] - lo).astype(np.int64)
        pos_k = pos[m].astype(np.int64)
        nbk, state = _bin_nodes(dl_k, pos_k)
        nb = max(nb, nbk)
        states.append((src_k, dl_k, pos_k, state))

    key = (nb, with_bias)
    if key not in _cache:
        _cache[key] = _build(nb, with_bias)
    nc = _cache[key]

    iota_t = np.ascontiguousarray(
        np.tile(np.arange(128, dtype=np.float32)[None, :], (128, 1)))

    def tile_param(v):
        return np.ascontiguousarray(np.tile(
            np.concatenate([np.asarray(v, np.float32)] * 2)[None, :], (128, 1)))

    rawl_t = tile_param(decomp_l)
    rawr_t = tile_param(decomp_r)

    in_maps = []
    metas = []
    for k in range(N_CORES):
        src_k, dl_k, pos_k, state = states[k]
        res, active, rows = _core_arrays(src_k, dl_k, pos_k, state, nb)
        im = {
            "feat": feat2,
            "gidxl": res["gidxl"], "gidxr": res["gidxr"],
            "rell": res["rell"], "relr": res["relr"],
            "iota": iota_t, "rawl": rawl_t, "rawr": rawr_t,
            "cnt": res["cnt"],
        }
        if with_bias:
            im["lbt"] = tile_param(lb)
            im["rbt"] = tile_param(rb)
            im["cntl"] = res["cntl"]
            im["cntr"] = res["cntr"]
        in_maps.append(im)
        metas.append((active, rows))

    if TRACE:
        _install_trace_hook()
    r = run_bass_kernel_spmd(nc, in_maps, list(range(N_CORES)), trace=TRACE)
    if TRACE:
        LAST_EXEC_NS = r.exec_time_ns
        LAST_RESULT = r

    full = np.zeros((N_NODES, EV), np.float32)
    for k in range(N_CORES):
        active, rows = metas[k]
        full[k * NPC + active] = r.results[k]["out"][rows]
    return full.reshape(N_NODES, 2, EV // 2)


def _install_trace_hook():
    import types
    if "antenv.axon_hooks" in sys.modules:
        return
    mod = types.ModuleType("antenv.axon_hooks")
    _hook = [None]
    mod.set_axon_ntff_profile_hook = lambda h: _hook.__setitem__(0, h)
    mod.get_axon_ntff_profile_hook = lambda: _hook[0]
    sys.modules["antenv.axon_hooks"] = mod
    try:
        import antenv
        antenv.axon_hooks = mod
        sys.path.insert(0, "/root/.axon_site/trn_agent_boot")
        import trn_boot
        mod.set_axon_ntff_profile_hook(
            trn_boot._ntff_profile_via_ctypes("/opt/axon/libaxon_pjrt.so"))
    except Exception:
        pass


# revision 23
# speedup vs baseline: 1.1036x; 1.1036x over previous
"""GNN message-passing (gather + pos-selected affine + segment mean) on 8 TRN2 cores.

Strategy: shard by destination node (2500 nodes/core; each core handles the edges
targeting its nodes — no cross-core reduction needed). All FP math on device:
  - dma_gather pulls feat[src] rows (1KB each) from the replicated HBM feat table.
  - The per-edge affine folds to per-node form: out[n] = (SL[n]*sig(l) + SR[n]*sig(r)
    + cntL[n]*lb + cntR[n]*rb) / max(cnt,1), where SL/SR are segment sums of the raw
    gathered rows split by pos. Segment sums run as one-hot matmuls (float32r, full
    PE rate) accumulating in PSUM over <=128-node windows.
Host does index-only preprocessing: partitions edges per core, bins nodes into
<=128-node windows with 512-edge-per-stream quotas, builds gather-index / relative
node-id arrays and integer degree counts, and un-permutes output rows at the end.
"""
import sys

_REPO = '/opt/trn_rl_repo'
if _REPO not in sys.path:
    sys.path.insert(0, _REPO)

from contextlib import ExitStack

import numpy as np
import concourse.bacc as bacc
import concourse.mybir as mybir
from concourse.bass_utils import run_bass_kernel_spmd
from concourse.library_config import mlp

N_NODES = 20000
N_CORES = 8
NPC = N_NODES // N_CORES      # nodes per core
EV = 256                      # flattened feature (2 * 128)
Q = 512                       # slots per window per stream (4 tiles of 128)
RD = 16                       # fp32r-converted data tile ring depth
RM = 32                       # one-hot M tile ring depth

TRACE = False
LAST_EXEC_NS = None
LAST_RESULT = None

_cache = {}


# ───────────────────────── device program ─────────────────────────

def _build(nb, with_bias):
    f32 = mybir.dt.float32
    f32r = mybir.dt.float32r
    i16 = mybir.dt.int16
    AF = mybir.ActivationFunctionType
    OP = mybir.AluOpType

    P = (nb + 3) // 4                       # gather chunks per stream
    wlist = [list(range(4 * p, min(4 * p + 4, nb))) for p in range(P)]
    S = nb * Q                              # slots per stream

    nc = bacc.Bacc("TRN2", target_bir_lowering=False, num_swdge_queues=4)

    feat = nc.dram_tensor("feat", [N_NODES, EV], f32, kind="ExternalInput")
    gidxl = nc.dram_tensor("gidxl", [128, S // 16], i16, kind="ExternalInput")
    gidxr = nc.dram_tensor("gidxr", [128, S // 16], i16, kind="ExternalInput")
    rell = nc.dram_tensor("rell", [128, S // 128], f32, kind="ExternalInput")
    relr = nc.dram_tensor("relr", [128, S // 128], f32, kind="ExternalInput")
    iota = nc.dram_tensor("iota", [128, 128], f32, kind="ExternalInput")
    rawl = nc.dram_tensor("rawl", [128, EV], f32, kind="ExternalInput")
    rawr = nc.dram_tensor("rawr", [128, EV], f32, kind="ExternalInput")
    cnt = nc.dram_tensor("cnt", [128, nb], f32, kind="ExternalInput")
    if with_bias:
        lbt = nc.dram_tensor("lbt", [128, EV], f32, kind="ExternalInput")
        rbt = nc.dram_tensor("rbt", [128, EV], f32, kind="ExternalInput")
        cntl = nc.dram_tensor("cntl", [128, nb], f32, kind="ExternalInput")
        cntr = nc.dram_tensor("cntr", [128, nb], f32, kind="ExternalInput")
    out = nc.dram_tensor("out", [nb * 128, EV], f32, kind="ExternalOutput")

    # dry pass: pt (PE tiles done) thresholds after each window's matmuls
    pt_L = {}
    pt_R = {}
    g = 0
    for p in range(P):
        for w in wlist[p]:
            g += 4
            pt_L[w] = g
        for w in wlist[p]:
            g += 4
            pt_R[w] = g
    # cv (fp32r converts done) thresholds after each chunk block, in Act order
    cv_L = {}
    cv_R = {}
    c = 0
    for p in range(P):
        c += 4 * len(wlist[p])
        cv_L[p] = c
        c += 4 * len(wlist[p])
        cv_R[p] = c

    n_in_dma = 6 + (4 if with_bias else 0)

    # dma_gather is limited to 1024 idxs (64 data descriptors per SDMA engine).
    # Sub-gathers round-robin across all 4 SWDGE queues (per-queue desc rings
    # serialize; spreading keeps 4 in flight). One sem per (stream, queue).
    GS = 8  # tiles per sub-gather
    gq_of = {}      # (stream, p, s0//GS) -> queue
    gcum_of = {}    # (stream, p) -> list of (queue, cum_count_after_chunk)
    qcnt = {('l', j): 0 for j in range(4)} | {('r', j): 0 for j in range(4)}
    sctr = {'l': 0, 'r': 0}
    for p in range(P):
        nt = 4 * len(wlist[p])
        for st in ('l', 'r'):
            for si, s0 in enumerate(range(0, nt, GS)):
                # reuse distance of 4 sub-gathers (2 chunks) per sem; chunk
                # p's gathers are gated on Act consuming chunk p-2, so no two
                # DMAs are ever in flight on one sem. R offset by 2 so both
                # streams cover all 4 queues each chunk pair.
                qq = (sctr[st] + (0 if st == 'l' else 2)) % 4
                sctr[st] += 1
                gq_of[(st, p, si)] = qq
                qcnt[(st, qq)] += 1
                gcum_of[(st, p, si)] = (qq, qcnt[(st, qq)])

    with ExitStack() as ctx:
        block = ctx.enter_context(nc.Block())
        sb = lambda name, shape, dt: ctx.enter_context(
            nc.sbuf_tensor(name, shape, dt))
        sem = lambda name: ctx.enter_context(nc.semaphore(name))

        gidxl_sb = sb("gidxl_sb", [128, S // 16], i16)
        gidxr_sb = sb("gidxr_sb", [128, S // 16], i16)
        rell_sb = sb("rell_sb", [128, S // 128], f32)
        relr_sb = sb("relr_sb", [128, S // 128], f32)
        iota_sb = sb("iota_sb", [128, 128], f32)
        rawl_sb = sb("rawl_sb", [128, EV], f32)
        rawr_sb = sb("rawr_sb", [128, EV], f32)
        sigl_sb = sb("sigl_sb", [128, EV], f32)
        sigr_sb = sb("sigr_sb", [128, EV], f32)
        cnt_sb = sb("cnt_sb", [128, nb], f32)
        cntm_sb = sb("cntm_sb", [128, nb], f32)
        recip_sb = sb("recip_sb", [128, nb], f32)
        bufl = [sb("bufl0", [128, 16, EV], f32), sb("bufl1", [128, 16, EV], f32)]
        bufr = [sb("bufr0", [128, 16, EV], f32), sb("bufr1", [128, 16, EV], f32)]
        data_r = sb("data_r", [128, RD, EV], f32r)
        m_sb = sb("m_sb", [128, RM, 128], f32r)
        slr_ring = sb("slr_ring", [128, 4, EV], f32)
        out_ring = sb("out_ring", [128, 4, EV], f32)
        tmp_sb = sb("tmp_sb", [128, EV], f32)
        if with_bias:
            lbt_sb = sb("lbt_sb", [128, EV], f32)
            rbt_sb = sb("rbt_sb", [128, EV], f32)
            cntl_sb = sb("cntl_sb", [128, nb], f32)
            cntr_sb = sb("cntr_sb", [128, nb], f32)
            cntlr_sb = sb("cntlr_sb", [128, nb], f32)
            cntrr_sb = sb("cntrr_sb", [128, nb], f32)

        # one PSUM bank ([128, 512] f32) per window slot; first 256 cols used
        psuml = [ctx.enter_context(nc.psum_tensor(f"psl{i}", [128, 512], f32))
                 for i in range(4)]
        psumr = [ctx.enter_context(nc.psum_tensor(f"psr{i}", [128, 512], f32))
                 for i in range(4)]

        io = sem("io")
        gsl = [sem(f"gsl{j}") for j in range(4)]
        gsr = [sem(f"gsr{j}") for j in range(4)]
        cv_sem = sem("cv")
        ms = sem("ms")
        pt = sem("pt")
        as_ = sem("as_")
        us = sem("us")
        af = sem("af")
        od = [sem("od0"), sem("od1"), sem("od2"), sem("od3")]
        rp = sem("rp")
        sg = sem("sg")
        aseq = sem("aseq")
        gix = sem("gix")
        wm = sem("wm")
        wmd = sem("wmd")

        @block.sync
        def _(sync):
            sync.dma_start(gidxl_sb[:], gidxl[:]).then_inc(gix, 16)
            sync.dma_start(gidxr_sb[:], gidxr[:]).then_inc(gix, 16)
            sync.dma_start(rell_sb[:], rell[:]).then_inc(io, 16)
            sync.dma_start(relr_sb[:], relr[:]).then_inc(io, 16)
            sync.dma_start(iota_sb[:], iota[:]).then_inc(io, 16)
            sync.dma_start(rawl_sb[:], rawl[:]).then_inc(io, 16)
            sync.dma_start(rawr_sb[:], rawr[:]).then_inc(io, 16)
            sync.dma_start(cnt_sb[:], cnt[:]).then_inc(io, 16)
            if with_bias:
                sync.dma_start(lbt_sb[:], lbt[:]).then_inc(io, 16)
                sync.dma_start(rbt_sb[:], rbt[:]).then_inc(io, 16)
                sync.dma_start(cntl_sb[:], cntl[:]).then_inc(io, 16)
                sync.dma_start(cntr_sb[:], cntr[:]).then_inc(io, 16)
            for w in range(nb):
                sync.wait_ge(af, w + 1)
                sync.dma_start(
                    out[w * 128:(w + 1) * 128, :], out_ring[:, w % 4, :]
                ).then_inc(od[w % 4], 16)
            for j in range(4):
                nd = len([w for w in range(nb) if w % 4 == j])
                sync.wait_ge(od[j], 16 * nd)

        @block.gpsimd
        def _(gpsimd):
            gpsimd.load_library(mlp)
            gpsimd.wait_ge(gix, 32)
            for p in range(P):
                nt = 4 * len(wlist[p])
                c0 = 32 * wlist[p][0]
                ncol = 8 * nt
                if p >= 2:
                    gpsimd.wait_ge(cv_sem, cv_L[p - 2])
                for si, s0 in enumerate(range(0, nt, GS)):
                    st = min(GS, nt - s0)
                    qq = gq_of[('l', p, si)]
                    gpsimd.dma_gather(
                        bufl[p % 2][:, s0:s0 + st, :], feat[:],
                        gidxl_sb[:, c0 + 8 * s0:c0 + 8 * (s0 + st)],
                        128 * st, 128 * st, EV, queue_num=qq,
                    ).then_inc(gsl[qq], 16)
                if p >= 2:
                    gpsimd.wait_ge(cv_sem, cv_R[p - 2])
                for si, s0 in enumerate(range(0, nt, GS)):
                    st = min(GS, nt - s0)
                    qq = gq_of[('r', p, si)]
                    gpsimd.dma_gather(
                        bufr[p % 2][:, s0:s0 + st, :], feat[:],
                        gidxr_sb[:, c0 + 8 * s0:c0 + 8 * (s0 + st)],
                        128 * st, 128 * st, EV, queue_num=qq,
                    ).then_inc(gsr[qq], 16)


        @block.scalar
        def _(scalar):
            scalar.wait_ge(io, 16 * n_in_dma)
            scalar.activation(sigl_sb[:], rawl_sb[:], AF.Sigmoid).then_inc(sg, 1)
            scalar.activation(sigr_sb[:], rawr_sb[:], AF.Sigmoid).then_inc(sg, 1)
            scalar.wait_ge(rp, 2)
            cvc = 0
            for p in range(P):
                nt = 4 * len(wlist[p])
                for si in range((nt + GS - 1) // GS):
                    jq, cnt_ = gcum_of[('l', p, si)]
                    scalar.wait_ge(gsl[jq], 16 * cnt_)
                for t0 in range(0, nt, 4):
                    if cvc + 4 > RD:
                        scalar.wait_ge(pt, cvc + 4 - RD)
                    scalar.copy(
                        data_r[:, cvc % RD:cvc % RD + 4, :],
                        bufl[p % 2][:, t0:t0 + 4, :],
                    ).then_inc(cv_sem, 4)
                    cvc += 4
                for si in range((nt + GS - 1) // GS):
                    jq, cnt_ = gcum_of[('r', p, si)]
                    scalar.wait_ge(gsr[jq], 16 * cnt_)
                for t0 in range(0, nt, 4):
                    if cvc + 4 > RD:
                        scalar.wait_ge(pt, cvc + 4 - RD)
                    scalar.copy(
                        data_r[:, cvc % RD:cvc % RD + 4, :],
                        bufr[p % 2][:, t0:t0 + 4, :],
                    ).then_inc(cv_sem, 4)
                    cvc += 4
                for w in wlist[p]:
                    if w >= 4:
                        scalar.wait_ge(af, w - 3)
                    scalar.wait_ge(pt, pt_L[w])
                    scalar.activation(
                        slr_ring[:, w % 4, :], psuml[w % 4][:, 0:EV], AF.Copy,
                        bias=0.0, scale=recip_sb[:, w:w + 1],
                    ).then_inc(as_, 1)

        @block.vector
        def _(vector):
            vector.wait_ge(io, 16 * n_in_dma)
            vector.tensor_scalar(cntm_sb[:], cnt_sb[:], 1.0, None, OP.max
                                 ).then_inc(rp, 1)
            vector.wait_ge(rp, 1)
            vector.reciprocal(recip_sb[:], cntm_sb[:]).then_inc(rp, 1)
            if with_bias:
                vector.wait_ge(rp, 2)
                vector.tensor_mul(cntlr_sb[:], cntl_sb[:], recip_sb[:]
                                  ).then_inc(rp, 1)
                vector.tensor_mul(cntrr_sb[:], cntr_sb[:], recip_sb[:]
                                  ).then_inc(rp, 1)

            ai = [0]
            av = [0]

            def affine(w):
                n = ai[0]
                vector.wait_ge(pt, pt_R[w])
                vector.wait_ge(as_, w + 1)
                vector.wait_ge(sg, 2)
                if w >= 4:
                    vector.wait_ge(od[w % 4], 16 * (w // 4))
                # u = (PSUM_R * recip) ⊙ sigR  -> out_ring slot
                vector.scalar_tensor_tensor(
                    out_ring[:, w % 4, :], psumr[w % 4][:, 0:EV],
                    recip_sb[:, w:w + 1], sigr_sb[:],
                    OP.mult, OP.mult,
                ).then_inc(us, 1)
                # v = SLr ⊙ sigL -> tmp (WAR vs previous window's add via af)
                if n >= 1:
                    vector.wait_ge(af, n)
                vector.tensor_mul(tmp_sb[:], slr_ring[:, w % 4, :], sigl_sb[:]
                                  ).then_inc(aseq, 1)
                av[0] += 1
                # out = u + v (same-engine RAW: both writes must retire)
                vector.wait_ge(us, n + 1)
                vector.wait_ge(aseq, av[0])
                last = vector.tensor_add(
                    out_ring[:, w % 4, :], tmp_sb[:], out_ring[:, w % 4, :])
                if with_bias:
                    last.then_inc(aseq, 1)
                    av[0] += 1
                    vector.wait_ge(rp, 4)
                    vector.wait_ge(aseq, av[0])
                    vector.scalar_tensor_tensor(
                        out_ring[:, w % 4, :], lbt_sb[:], cntlr_sb[:, w:w + 1],
                        out_ring[:, w % 4, :], OP.mult, OP.add
                    ).then_inc(aseq, 1)
                    av[0] += 1
                    vector.wait_ge(aseq, av[0])
                    last = vector.scalar_tensor_tensor(
                        out_ring[:, w % 4, :], rbt_sb[:], cntrr_sb[:, w:w + 1],
                        out_ring[:, w % 4, :], OP.mult, OP.add)
                last.then_inc(af, 1)
                ai[0] = n + 1

            mcv = [0]
            iota_b = iota_sb[:, None, :].broadcast_to([128, 4, 128])

            def mbuild(rel_tab, p, t0):
                mc = mcv[0]
                col = 4 * wlist[p][0] + t0
                if mc + 4 > RM:
                    vector.wait_ge(pt, mc + 4 - RM)
                rel_b = rel_tab[:, col:col + 4][:, :, None].broadcast_to(
                    [128, 4, 128])
                vector.tensor_tensor(
                    m_sb[:, mc % RM:mc % RM + 4, :], iota_b, rel_b,
                    OP.is_equal,
                ).then_inc(ms, 4)

            for p in range(P):
                nt = 4 * len(wlist[p])
                for t0 in range(0, nt, 4):
                    mbuild(rell_sb, p, t0)
                    mcv[0] += 4
                if p >= 1:
                    for w in wlist[p - 1]:
                        affine(w)
                for t0 in range(0, nt, 4):
                    mbuild(relr_sb, p, t0)
                    mcv[0] += 4
            for w in wlist[P - 1]:
                affine(w)

        @block.tensor
        def _(tensor):
            gg = 0
            for p in range(P):
                for w in wlist[p]:
                    if w >= 4:
                        tensor.wait_ge(as_, w - 3)
                    for q_ in range(4):
                        tensor.wait_ge(cv_sem, gg - gg % 4 + 4)
                        tensor.wait_ge(ms, gg - gg % 4 + 4)
                        tensor.matmul(
                            psuml[w % 4][:, 0:EV], m_sb[:, gg % RM, :],
                            data_r[:, gg % RD, :],
                            start=(q_ == 0), stop=(q_ == 3),
                        ).then_inc(pt, 1)
                        gg += 1
                for w in wlist[p]:
                    if w >= 4:
                        tensor.wait_ge(us, w - 3)
                    for q_ in range(4):
                        tensor.wait_ge(cv_sem, gg - gg % 4 + 4)
                        tensor.wait_ge(ms, gg - gg % 4 + 4)
                        tensor.matmul(
                            psumr[w % 4][:, 0:EV], m_sb[:, gg % RM, :],
                            data_r[:, gg % RD, :],
                            start=(q_ == 0), stop=(q_ == 3),
                        ).then_inc(pt, 1)
                        gg += 1

    nc.compile()
    return nc


# ───────────────────────── host preprocessing ─────────────────────────

def _wrap16(idx):
    a = idx.reshape(-1, 16).T.astype(np.int16)
    return np.ascontiguousarray(np.tile(a, (8, 1)))


def _bin_nodes(dl_k, pos_k):
    degL = np.bincount(dl_k[pos_k == 0], minlength=NPC)
    degR = np.bincount(dl_k[pos_k == 1], minlength=NPC)
    deg = degL + degR
    active = np.where(deg > 0)[0]
    order = active[np.argsort(-deg[active], kind='stable')]

    node_bin = np.full(NPC, -1, np.int64)
    node_rel = np.full(NPC, -1, np.int64)
    bins = []  # [node_count, l_sum, r_sum]
    for n in order:
        dn_l, dn_r = int(degL[n]), int(degR[n])
        for bi, b in enumerate(bins):
            if b[0] < 128 and b[1] + dn_l <= Q and b[2] + dn_r <= Q:
                node_bin[n] = bi
                node_rel[n] = b[0]
                b[0] += 1
                b[1] += dn_l
                b[2] += dn_r
                break
        else:
            node_bin[n] = len(bins)
            node_rel[n] = 0
            bins.append([1, dn_l, dn_r])
    return len(bins), (degL, degR, deg, active, node_bin, node_rel)


def _core_arrays(src_k, dl_k, pos_k, state, nb):
    degL, degR, deg, active, node_bin, node_rel = state
    S = nb * Q
    res = {}
    for name, sel in (("l", pos_k == 0), ("r", pos_k == 1)):
        es = np.where(sel)[0]
        b = node_bin[dl_k[es]]
        o = np.lexsort((src_k[es], b))
        se = es[o]
        bb = node_bin[dl_k[se]]
        starts = np.searchsorted(bb, np.arange(nb))
        rank = np.arange(len(se)) - starts[bb]
        slot = bb * Q + rank
        gidx = np.zeros(S, np.int64)
        rel = np.full(S, -1.0, np.float32)
        gidx[slot] = src_k[se]
        rel[slot] = node_rel[dl_k[se]]
        res["gidx" + name] = _wrap16(gidx)
        res["rel" + name] = np.ascontiguousarray(rel.reshape(S // 128, 128).T)
    for nm, d in (("cnt", deg), ("cntl", degL), ("cntr", degR)):
        t = np.zeros((128, nb), np.float32)
        t[node_rel[active], node_bin[active]] = d[active]
        res[nm] = t
    rows = node_bin[active] * 128 + node_rel[active]
    return res, active, rows


# ───────────────────────── entry point ─────────────────────────

def kernel(feat, decomp_l, decomp_r, lb, rb, src, dst, pos):
    global LAST_EXEC_NS, LAST_RESULT
    feat = np.asarray(feat, np.float32)
    src = np.asarray(src)
    dst = np.asarray(dst)
    pos = np.asarray(pos)
    feat2 = np.ascontiguousarray(feat.reshape(N_NODES, EV))
    with_bias = bool(np.any(np.asarray(lb)) or np.any(np.asarray(rb)))

    states = []
    nb = 0
    for k in range(N_CORES):
        lo = k * NPC
        m = (dst >= lo) & (dst < lo + NPC)
        src_k = src[m].astype(np.int64)
        dl_k = (dst[m] - lo).astype(np.int64)
        pos_k = pos[m].astype(np.int64)
        nbk, state = _bin_nodes(dl_k, pos_k)
        nb = max(nb, nbk)
        states.append((src_k, dl_k, pos_k, state))

    key = (nb, with_bias)
    if key not in _cache:
        _cache[key] = _build(nb, with_bias)
    nc = _cache[key]

    iota_t = np.ascontiguousarray(
        np.tile(np.arange(128, dtype=np.float32)[None, :], (128, 1)))

    def tile_param(v):
        return np.ascontiguousarray(np.tile(
            np.concatenate([np.asarray(v, np.float32)] * 2)[None, :], (128, 1)))

    rawl_t = tile_param(decomp_l)
    rawr_t = tile_param(decomp_r)

    in_maps = []
    metas = []
    for k in range(N_CORES):
        src_k, dl_k, pos_k, state = states[k]
        res, active, rows = _core_arrays(src_k, dl_k, pos_k, state, nb)
        im = {
            "feat": feat2,
            "gidxl": res["gidxl"], "gidxr": res["gidxr"],
            "rell": res["rell"], "relr": res["relr"],
            "iota": iota_t, "rawl": rawl_t, "rawr": rawr_t,
            "cnt": res["cnt"],
        }
        if with_bias:
            im["lbt"] = tile_param(lb)
            im["rbt"] = tile_param(rb)
            im["cntl"] = res["cntl"]
            im["cntr"] = res["cntr"]
        in_maps.append(im)
        metas.append((active, rows))

    if TRACE:
        _install_trace_hook()
    r = run_bass_kernel_spmd(nc, in_maps, list(range(N_CORES)), trace=TRACE)
    if TRACE:
        LAST_EXEC_NS = r.exec_time_ns
        LAST_RESULT = r

    full = np.zeros((N_NODES, EV), np.float32)
    for k in range(N_CORES):
        active, rows = metas[k]
        full[k * NPC + active] = r.results[k]["out"][rows]
    return full.reshape(N_NODES, 2, EV // 2)


def _install_trace_hook():
    import types
    if "antenv.axon_hooks" in sys.modules:
        return
    mod = types.ModuleType("antenv.axon_hooks")
    _hook = [None]
    mod.set_axon_ntff_profile_hook = lambda h: _hook.__setitem__(0, h)
    mod.get_axon_ntff_profile_hook = lambda: _hook[0]
    sys.modules["antenv.axon_hooks"] = mod
    try:
        import antenv
        antenv.axon_hooks = mod
        sys.path.insert(0, "/root/.axon_site/trn_agent_boot")
        import trn_boot
        mod.set_axon_ntff_profile_hook(
            trn_boot._ntff_profile_via_ctypes("/opt/axon/libaxon_pjrt.so"))
    except Exception:
        pass
